# revision 5
# baseline (speedup 1.0000x reference)
"""Trainium2 Bass kernel for ExportableCostVolume (cross-correlation cost volume).

out[b, d, h, w] = mean_c left[b,c,h,w] * right[b,c,h,w-d]   (w >= d, else 0)
B=4, C=128, H=256, W=512, D=128.

Strategy (8 NeuronCores, sharded over H):
  Per (b, h) image row, per 128-wide w-block j (stationary = left cols):
    G[wi, v] = sum_c L[c, w0+wi] * R[c, w0+127-v]        (TensorE, fp32r, moving
                reads the R window reversed so the band comes out with a
                *negative* per-partition drift)
  Then out[d, w0+wi] = G[wi, 127-wi+d]: per SBUF partition wi this is a
  contiguous 512B run starting at per-partition offset 127-wi.  A SWDGE
  SBUF->SBUF DMA with a flat "diagonal" access pattern (partition step =
  row_len-1 elements) extracts the whole band at full fabric rate into
  bt[wi, d] = out[d, w0+wi], which a TensorE transpose turns into the final
  (d, w) layout for contiguous HBM writes.
"""
import sys

sys.path.insert(0, "/opt/trn_rl_repo")

import numpy as np

import concourse.bass as bass
import concourse.mybir as mybir
import concourse.tile as tile
from concourse import bacc
from concourse.bass_utils import run_bass_kernel_spmd
from concourse.masks import make_identity

B, C, H, W, D = 4, 128, 256, 512, 128
NCORES = 8
HPC = H // NCORES  # h rows per core
HB = 4             # h rows per pipeline batch
NJ = W // 128      # w-blocks per row

_nc_cache = None


def _build_nc(iters: int = 1):
    fr = mybir.dt.float32r
    f32 = mybir.dt.float32

    nc = bacc.Bacc()
    left_s = nc.declare_dram_parameter("left", [B, C, HPC, W], f32, isOutput=False)
    right_s = nc.declare_dram_parameter("right", [B, C, HPC, W], f32, isOutput=False)
    out_s = nc.declare_dram_parameter("out", [B, D, HPC, W], f32, isOutput=True)

    nbatch = B * (HPC // HB) * iters

    def batch_src(t):
        b, hb = divmod(t % (B * (HPC // HB)), HPC // HB)
        return b, hb * HB

    with tile.TileContext(nc) as tc:
        with (
            tc.tile_pool(name="consts", bufs=1) as consts,
            tc.tile_pool(name="pool", bufs=2) as pool,
            tc.tile_pool(name="ps", bufs=4, space="PSUM") as ps,
            tc.tile_pool(name="ps2", bufs=4, space="PSUM") as ps2,
        ):
            ident = consts.tile([128, 128], f32)
            make_identity(nc, ident)

            stage = {}  # t -> (bt4, Osb4, b, h0)

            for t in range(nbatch + 1):
                if t < nbatch:
                    b, h0 = batch_src(t)
                    Ls4 = pool.tile([128, HB, W], fr, tag="Ls4")
                    Rs4 = pool.tile([128, HB, W], fr, tag="Rs4")
                    Gsb4 = pool.tile([128, HB, 1024], f32, tag="Gsb4")
                    bt4 = pool.tile([128, HB, 512], f32, tag="bt4")
                    # input loads (SWDGE: casts fp32 -> fp32r)
                    nc.gpsimd.dma_start(Ls4[:], left_s[b, :, h0:h0 + HB, :])
                    nc.gpsimd.dma_start(Rs4[:], right_s[b, :, h0:h0 + HB, :])
                    # zero fill for the w<d corner of block j=0
                    nc.vector.memset(Gsb4[:, :, 128:256], 0.0)
                    for h in range(HB):
                        for j in range(NJ):
                            g = ps.tile([128, 256], f32, tag="g")
                            # moving operand: R cols (h, u), u descending from ustart
                            ustart = 255 if j == 0 else j * 128 + 127
                            rev = bass.AP(
                                Rs4.tensor,
                                Rs4.offset + h * W + ustart,
                                [[HB * W, 128], [-1, 256]],
                            )
                            nc.tensor.matmul(
                                g[:], Ls4[:, h, j * 128:(j + 1) * 128], rev,
                                start=True, stop=True,
                            )
                            # copy band half to SBUF with the 1/C mean scaling
                            gsrc = g[:, 128:256] if j == 0 else g[:]
                            gdst = (
                                Gsb4[:, h, 0:128] if j == 0
                                else Gsb4[:, h, j * 256:j * 256 + 256]
                            )
                            if (h * NJ + j) % 2 == 0:
                                nc.scalar.mul(gdst, gsrc, 1.0 / C)
                            else:
                                nc.vector.tensor_scalar_mul(gdst, gsrc, 1.0 / C)
                    # diagonal band extraction: bt4[wi, h, j*128+d] =
                    #   Gsb4[wi, h, 256j + 127 - wi + d]
                    src = bass.AP(
                        Gsb4.tensor,
                        Gsb4.offset + 127,
                        [[HB * 1024 - 1, 128], [1024, HB], [256, NJ], [1, 128]],
                    )
                    nc.gpsimd.dma_start(bt4[:], src)
                    stage[t] = (bt4, b, h0)

                if t >= 1:
                    bt4, b, h0 = stage.pop(t - 1)
                    Osb4 = pool.tile([128, HB, W], f32, tag="Osb4")
                    for h in range(HB):
                        for j in range(NJ):
                            tp = ps2.tile([128, 128], f32, tag="tp")
                            nc.tensor.transpose(
                                tp[:], bt4[:, h, j * 128:(j + 1) * 128], ident[:]
                            )
                            if (h * NJ + j) % 2 == 0:
                                nc.vector.tensor_copy(
                                    Osb4[:, h, j * 128:(j + 1) * 128], tp[:]
                                )
                            else:
                                nc.scalar.copy(
                                    Osb4[:, h, j * 128:(j + 1) * 128], tp[:]
                                )
                    nc.sync.dma_start(out_s[b, :, h0:h0 + HB, :], Osb4[:])

    nc.finalize()
    return nc


def kernel(left: np.ndarray, right: np.ndarray, _iters: int = 1) -> np.ndarray:
    global _nc_cache
    if _nc_cache is None:
        _nc_cache = {}
    if _iters not in _nc_cache:
        _nc_cache[_iters] = _build_nc(_iters)
    nc = _nc_cache[_iters]

    left = np.ascontiguousarray(left, dtype=np.float32)
    right = np.ascontiguousarray(right, dtype=np.float32)
    in_maps = []
    for k in range(NCORES):
        sl = slice(k * HPC, (k + 1) * HPC)
        in_maps.append({
            "left": np.ascontiguousarray(left[:, :, sl, :]),
            "right": np.ascontiguousarray(right[:, :, sl, :]),
        })
    res = run_bass_kernel_spmd(nc, in_maps, list(range(NCORES)))
    out = np.concatenate([res.results[k]["out"] for k in range(NCORES)], axis=2)
    return out


# revision 12
# speedup vs baseline: 18.4191x; 18.4191x over previous
"""Trainium2 Bass kernel for ExportableCostVolume (cross-correlation cost volume).

out[b, d, h, w] = mean_c left[b,c,h,w] * right[b,c,h,w-d]   (w >= d, else 0)
B=4, C=128, H=256, W=512, D=128.

Strategy (8 NeuronCores, data-parallel over H stripes):
  Per (b, h) image row, per 128-wide w-block j (stationary = left cols):
    G[wi, v] = sum_c L[c, w0+wi] * R[c, w0+127-v]      (TensorE; the moving
               operand reads the R window reversed, so the output band sits on
               anti-diagonals with a *negative* per-partition drift)
  Then out[d, w0+wi] = G[wi, 127-wi+d]: per SBUF partition wi this is one
  contiguous run starting at per-partition offset 127-wi.  An HWDGE
  SBUF->SBUF DMA with a flat "diagonal" access pattern (partition step =
  row_len-1 elements, contiguous inner dim) extracts the whole band at
  fabric rate into bt[wi, d] = out[d, w0+wi]; a TensorE transpose then yields
  the final (d, w) layout for contiguous HBM writes.

DTYPE_MODE:
  "fp32r": TF32-like matmul at full PE rate, ~1.6e-4 scale-relative error.
  "bf16":  bf16 inputs (pre-cast on host) + bf16 band staging, ~3.7e-3 error,
           roughly half the DMA traffic.
"""
import sys

sys.path.insert(0, "/opt/trn_rl_repo")

import ml_dtypes
import numpy as np

import concourse.bass as bass
import concourse.mybir as mybir
import concourse.tile as tile
from concourse import bacc
from concourse.bass_utils import run_bass_kernel_spmd
from concourse.masks import make_identity

B, C, H, W, D = 4, 128, 256, 512, 128
NCORES = 8
HPC = H // NCORES  # h rows per core
HB = 4             # h rows per pipeline batch
NJ = W // 128      # w-blocks per row

DTYPE_MODE = "fp32r"

_nc_cache = {}


def _build_nc(iters: int = 1, mode: str | None = None):
    mode = mode or DTYPE_MODE
    bf16 = mode == "bf16"
    fr = mybir.dt.bfloat16 if bf16 else mybir.dt.float32r
    fband = mybir.dt.bfloat16 if bf16 else mybir.dt.float32
    f32 = mybir.dt.float32

    nc = bacc.Bacc()
    left_s = nc.declare_dram_parameter("left", [B, C, HPC, W], fr, isOutput=False)
    right_s = nc.declare_dram_parameter("right", [B, C, HPC, W], fr, isOutput=False)
    out_s = nc.declare_dram_parameter("out", [B, D, HPC, W], f32, isOutput=True)

    nbatch = B * (HPC // HB) * iters

    def batch_src(t):
        b, hb = divmod(t % (B * (HPC // HB)), HPC // HB)
        return b, hb * HB

    with tile.TileContext(nc) as tc:
        with (
            tc.tile_pool(name="consts", bufs=1) as consts,
            tc.tile_pool(name="pool", bufs=2) as pool,
            tc.tile_pool(name="ps", bufs=4, space="PSUM") as ps,
            tc.tile_pool(name="ps2", bufs=4, space="PSUM") as ps2,
        ):
            ident = consts.tile([128, 128], fband)
            make_identity(nc, ident)

            stage = {}  # t -> (bt4, b, h0)

            for t in range(nbatch + 1):
                if t < nbatch:
                    b, h0 = batch_src(t)
                    Ls4 = pool.tile([128, HB, W], fr, tag="Ls4")
                    Rs4 = pool.tile([128, HB, W], fr, tag="Rs4")
                    Gsb4 = pool.tile([128, HB, 1024], fband, tag="Gsb4")
                    bt4 = pool.tile([128, HB, 512], fband, tag="bt4")
                    # input loads (SWDGE: spreads descriptors over 16 engines,
                    # measured faster than HWDGE for these strided patterns)
                    nc.gpsimd.dma_start(Ls4[:], left_s[b, :, h0:h0 + HB, :])
                    nc.gpsimd.dma_start(Rs4[:], right_s[b, :, h0:h0 + HB, :])
                    # zero fill for the w<d corner of block j=0
                    nc.vector.memset(Gsb4[:, :, 128:256], 0.0)
                    for h in range(HB):
                        for j in range(NJ):
                            g = ps.tile([128, 256], f32, tag="g")
                            # moving operand: R cols (h, u), u descending from ustart
                            ustart = 255 if j == 0 else j * 128 + 127
                            rev = bass.AP(
                                Rs4.tensor,
                                Rs4.offset + h * W + ustart,
                                [[HB * W, 128], [-1, 256]],
                            )
                            nc.tensor.matmul(
                                g[:], Ls4[:, h, j * 128:(j + 1) * 128], rev,
                                start=True, stop=True,
                            )
                            # copy band half to SBUF with the 1/C mean scaling
                            gsrc = g[:, 128:256] if j == 0 else g[:]
                            gdst = (
                                Gsb4[:, h, 0:128] if j == 0
                                else Gsb4[:, h, j * 256:j * 256 + 256]
                            )
                            if (h * NJ + j) % 2 == 0:
                                nc.scalar.mul(gdst, gsrc, 1.0 / C)
                            else:
                                nc.vector.tensor_scalar_mul(gdst, gsrc, 1.0 / C)
                    # diagonal band extraction (SWDGE SBUF->SBUF):
                    #   bt4[wi, h, j*128+d] = Gsb4[wi, h, 256j + 127 - wi + d]
                    src = bass.AP(
                        Gsb4.tensor,
                        Gsb4.offset + 127,
                        [[HB * 1024 - 1, 128], [1024, HB], [256, NJ], [1, 128]],
                    )
                    nc.gpsimd.dma_start(bt4[:], src)
                    stage[t] = (bt4, b, h0)

                if t >= 1:
                    bt4, b, h0 = stage.pop(t - 1)
                    Osb4 = pool.tile([128, HB, W], f32, tag="Osb4")
                    for h in range(HB):
                        for j in range(NJ):
                            tp = ps2.tile([128, 128], fband, tag="tp")
                            nc.tensor.transpose(
                                tp[:], bt4[:, h, j * 128:(j + 1) * 128], ident[:]
                            )
                            if (h * NJ + j) % 2 == 0:
                                nc.vector.tensor_copy(
                                    Osb4[:, h, j * 128:(j + 1) * 128], tp[:]
                                )
                            else:
                                nc.scalar.copy(
                                    Osb4[:, h, j * 128:(j + 1) * 128], tp[:]
                                )
                    nc.sync.dma_start(out_s[b, :, h0:h0 + HB, :], Osb4[:])

    nc.finalize()
    return nc


def kernel(left: np.ndarray, right: np.ndarray, _iters: int = 1) -> np.ndarray:
    key = (_iters, DTYPE_MODE)
    if key not in _nc_cache:
        _nc_cache[key] = _build_nc(_iters)
    nc = _nc_cache[key]

    in_dt = ml_dtypes.bfloat16 if DTYPE_MODE == "bf16" else np.float32
    left = np.asarray(left, dtype=np.float32).astype(in_dt)
    right = np.asarray(right, dtype=np.float32).astype(in_dt)
    in_maps = []
    for k in range(NCORES):
        sl = slice(k * HPC, (k + 1) * HPC)
        in_maps.append({
            "left": np.ascontiguousarray(left[:, :, sl, :]),
            "right": np.ascontiguousarray(right[:, :, sl, :]),
        })
    res = run_bass_kernel_spmd(nc, in_maps, list(range(NCORES)))
    out = np.concatenate([res.results[k]["out"] for k in range(NCORES)], axis=2)
    return out
